# revision 11
# baseline (speedup 1.0000x reference)
"""Trainium2 Bass kernel for batched tanh-attention flat-softmax.

Per batch b:
    Q = query[b] @ W_query; K = query[b] @ W_key      # [S, 64]
    s = tanh(Q @ K.T) * 10                            # [S, S]
    s[diag] = -inf
    out[b] = softmax(s.flatten())

Sharding: data-parallel over batch across 8 NeuronCores (6 batches/core),
W_query/W_key replicated; no cross-core communication.

Numerics: tanh(x)*10 is bounded in [-10,10], so softmax needs no max
subtraction: out = exp(10*tanh(s)) / sum(...). The diagonal is clamped to
-1e4 on the tanh output (post-tanh, on SBUF, so the clamp is never on the
PE->tanh critical path), so exp(10*-1e4) underflows to exactly 0, matching
the reference's additive -1e8 mask.

Precision strategy (validated vs fp32 reference: rel L2 ~6e-3):
  - query is rounded once to bf16 (xh); its lo half is dropped.
  - W is kept as bf16 hi+lo ([Wq|Wk] stacked): proj = wh.T xhT + wl.T xhT
    accumulated in fp32 PSUM -> pp = [Q; K] with k on partitions.
  - scores = Qh.T Kh in pure bf16 (64-contraction), no correction terms.
  - xhT is produced by PE transposes (identity matmul) into a bf16 PSUM
    region -- no DRAM-roundtrip DMA transpose.

PSUM layout: score pool of 3 x [P, S] fp32 chunk tiles (2 banks each) +
one [P, 512] fp32 front-end tile (2 banks: proj accumulator in fp32 cols
0:512 bank 0; PE-transpose scratch in a bf16 bitcast view, bank 1) = 8
banks. Separate pool tiles get precise dependency tracking (slices of one
big tensor serialize conservatively). Slot-reuse distance 3 on the score
ring gives ~3.3us of tanh slack per matmul, so the PE never stalls the
scalar engine; the scalar engine streams tanh/exp back-to-back.

Engine budget per core (target): scalar ~97us (tanh+exp = roofline),
DMA ~78us (24MB output store), PE ~50us, DVE ~55us.
"""

import numpy as np

import concourse.bass as bass
import concourse.bass_isa as bass_isa
import concourse.mybir as mybir
import concourse.tile as tile
from concourse import bacc
from concourse.bass_utils import run_bass_kernel_spmd

B = 48
S = 1024
D = 128
DK = 64
N_CORES = 8
BPC = B // N_CORES
P = 128
NQ = S // P
F32 = mybir.dt.float32
BF16 = mybir.dt.bfloat16
AL = mybir.AluOpType

TANH_CLIP = 10.0
DIAG_NEG = -1.0e4  # post-tanh diag clamp; exp(10 * -1e4) == 0 exactly


def build_bass() -> bass.Bass:
    nc = bacc.Bacc(None, target_bir_lowering=False)

    q_d = nc.dram_tensor("query", [BPC, S, D], F32, kind="ExternalInput")
    wq_d = nc.dram_tensor("W_query", [D, DK], F32, kind="ExternalInput")
    wk_d = nc.dram_tensor("W_key", [D, DK], F32, kind="ExternalInput")
    out_d = nc.dram_tensor("out", [BPC, S, S], F32, kind="ExternalOutput")

    with tile.TileContext(nc) as tc:
        with (
            tc.tile_pool(name="singles", bufs=1) as singles,
            tc.tile_pool(name="qload", bufs=3) as qload,
            tc.tile_pool(name="xhp", bufs=2) as xhp,
            tc.tile_pool(name="xhtp", bufs=2) as xhtp,
            tc.tile_pool(name="hbp", bufs=2) as hbp,
            tc.tile_pool(name="tbuf", bufs=3) as tbuf,
            tc.tile_pool(name="small", bufs=2) as small,
            tc.tile_pool(name="ps", bufs=3, space="PSUM") as ps,
            tc.tile_pool(name="psfe", bufs=1, space="PSUM") as psfe,
        ):
            # --- one-time setup ---
            # warm the exp/tanh activation table set during the initial DMAs
            warm = singles.tile([P, 1], F32)
            nc.vector.memset(warm, 0.0)
            nc.scalar.activation(
                out=warm, in_=warm, func=mybir.ActivationFunctionType.Tanh
            )

            def load_q(b):
                q_sb = qload.tile([P, NQ, D], F32, tag="q")
                nc.sync.dma_start(
                    q_sb, q_d[b].rearrange("(n p) d -> p n d", p=P)
                )
                return q_sb

            # query loads go out first: the front-end consumes them earliest
            q_loaded = [load_q(0), load_q(1)]

            # bf16 identity for PE transposes
            ident = singles.tile([P, P], BF16)
            nc.vector.memset(ident, 0.0)
            nc.gpsimd.affine_select(
                out=ident,
                in_=ident,
                compare_op=AL.not_equal,
                fill=1.0,
                base=0,
                pattern=[[-1, P]],
                channel_multiplier=1,
            )
            # diag clamp mask: min(tanh_out, dmask) forces diagonal to -1e4
            dmask = singles.tile([P, P], F32)
            nc.vector.memset(dmask, 3.0e38)
            nc.gpsimd.affine_select(
                out=dmask,
                in_=dmask,
                compare_op=AL.not_equal,
                fill=DIAG_NEG,
                base=0,
                pattern=[[-1, P]],
                channel_multiplier=1,
            )

            # W stacked [Wq | Wk] as fp32, then bf16 hi/lo
            w32 = singles.tile([D, 2 * DK], F32)
            nc.sync.dma_start(w32[:, 0:DK], wq_d[:, :])
            nc.sync.dma_start(w32[:, DK:2 * DK], wk_d[:, :])
            wh = singles.tile([D, 2 * DK], BF16)
            nc.vector.tensor_copy(wh, w32)
            wl = singles.tile([D, 2 * DK], BF16)
            nc.vector.tensor_tensor(wl, w32, wh, AL.subtract)

            # persistent 2-bank front-end PSUM tile: proj accumulator in
            # fp32 cols 0:512 (bank 0), PE-transpose scratch as bf16 view
            # of fp32 cols 512:1024 (bank 1)
            fe = psfe.tile([P, S], F32)
            pp = fe[:, 0:512]
            tp = fe.bitcast(BF16)[:, S:2 * S]

            def front_end_a(q_sb):
                """Cast query to bf16, PE-transpose all 8 chunks -> xhT."""
                xh = xhp.tile([P, NQ, D], BF16, tag="xh")
                nc.vector.tensor_copy(xh, q_sb)
                for c in range(NQ):
                    nc.tensor.transpose(
                        tp[:, c * P:(c + 1) * P], xh[:, c, :], ident
                    )
                xhT = xhtp.tile([P, S], BF16, tag="xhT")
                nc.vector.tensor_copy(xhT, tp)
                return xhT

            def front_end_b(xhT):
                """proj pp = [Q;K] (fp32 psum) -> hb bf16; kh = dup of Kh.
                hb/kh are split into per-half tiles so a chunk matmul's
                dependency resolves as soon as its own half is cast."""
                hbs, khs = [], []
                for h in range(2):
                    cols = slice(h * 512, (h + 1) * 512)
                    hb = hbp.tile([P, 512], BF16, tag=f"hb{h}")
                    kh = hbp.tile([P, 512], BF16, tag=f"kh{h}")
                    nc.tensor.matmul(pp, wh, xhT[:, cols], start=True, stop=False)
                    nc.tensor.matmul(pp, wl, xhT[:, cols], start=False, stop=True)
                    nc.vector.tensor_copy(hb, pp)
                    nc.vector.tensor_copy(kh[0:DK, :], hb[DK:P, :])
                    hbs.append(hb)
                    khs.append(kh)
                return hbs, khs

            def score_tanh_chunk(hbs, khs, t_sb, c):
                """Chunk c: 2 matmuls into a [P, S] PSUM ring tile, then
                tanh straight to t_sb. Nothing but the PE feeds the tanh."""
                t = ps.tile([P, S], F32, tag="sc")
                # chunk c's Q columns live in half c//4 of hb
                lhsT = hbs[c // 4][0:DK, (c % 4) * P:(c % 4 + 1) * P]
                for h in range(2):
                    cols = slice(h * 512, (h + 1) * 512)
                    nc.tensor.matmul(
                        t[:, cols], lhsT, khs[h][0:DK, :], start=True, stop=True
                    )
                nc.scalar.activation(
                    out=t_sb[:, c],
                    in_=t,
                    func=mybir.ActivationFunctionType.Tanh,
                )

            def diag_clamp_pair(t_sb, j):
                """Clamp diag blocks of chunks 2j, 2j+1 on the tanh output
                (SBUF) with one strided min; off the tanh critical path,
                only exp depends on it."""
                blk0 = t_sb[:, 2 * j, 2 * j * P:(2 * j + 1) * P]
                diag_ap = bass.AP(
                    tensor=blk0.tensor,
                    offset=blk0.offset,
                    ap=[blk0.ap[0], [S + P, 2], [1, P]],
                )
                m0 = dmask[:, 0:P]
                mask_ap = bass.AP(
                    tensor=m0.tensor,
                    offset=m0.offset,
                    ap=[m0.ap[0], [0, 2], [1, P]],
                )
                nc.vector.tensor_tensor(diag_ap, diag_ap, mask_ap, AL.min)

            def exp_batch(t_sb):
                """exp(10*t) in place over the whole batch, row sums -> rs."""
                rs = small.tile([P, 1], F32, tag="rs")
                nc.scalar.activation(
                    out=t_sb,
                    in_=t_sb,
                    func=mybir.ActivationFunctionType.Exp,
                    scale=TANH_CLIP,
                    accum_out=rs,
                )
                zall = small.tile([P, 1], F32, tag="zall")
                nc.gpsimd.partition_all_reduce(
                    zall, rs, channels=P, reduce_op=bass_isa.ReduceOp.add
                )
                rz = small.tile([P, 1], F32, tag="rz")
                nc.vector.reciprocal(rz, zall)
                return rz

            def norm_chunks(t_sb, rz, c0, n):
                nc.vector.tensor_scalar_mul(
                    t_sb[:, c0:c0 + n], t_sb[:, c0:c0 + n], rz
                )

            def store_chunks(b, t_sb, c0, n):
                """store via SWDGE on the (otherwise idle) GpSimd queue"""
                nc.gpsimd.dma_start(
                    out_d[b].rearrange("(n p) s -> p n s", p=P)[:, c0:c0 + n],
                    t_sb[:, c0:c0 + n],
                )

            # ---- software-pipelined batch loop --------------------------
            # prologue: batch 0 front-end (q0/q1 loads already in flight)
            xhT = front_end_a(q_loaded[0])
            ops = front_end_b(xhT)
            pending = None  # (b, t_sb, rz) awaiting normalize+store

            for b in range(BPC):
                t_sb = tbuf.tile([P, NQ, S], F32, tag="t")
                hbs, khs = ops

                if b + 2 < BPC:
                    q_loaded.append(load_q(b + 2))

                score_tanh_chunk(hbs, khs, t_sb, 0)
                score_tanh_chunk(hbs, khs, t_sb, 1)
                diag_clamp_pair(t_sb, 0)
                # next batch front-end part A (cast + PE transposes)
                if b + 1 < BPC:
                    xhT_next = front_end_a(q_loaded[b + 1])

                score_tanh_chunk(hbs, khs, t_sb, 2)
                score_tanh_chunk(hbs, khs, t_sb, 3)
                diag_clamp_pair(t_sb, 1)
                # next batch front-end part B (proj + hb/kh)
                if b + 1 < BPC:
                    ops = front_end_b(xhT_next)

                score_tanh_chunk(hbs, khs, t_sb, 4)
                score_tanh_chunk(hbs, khs, t_sb, 5)
                diag_clamp_pair(t_sb, 2)

                score_tanh_chunk(hbs, khs, t_sb, 6)
                score_tanh_chunk(hbs, khs, t_sb, 7)
                diag_clamp_pair(t_sb, 3)

                rz = exp_batch(t_sb)

                # previous batch's normalize + store: emitted after exp so
                # this batch's diag clamps are never queued behind the 4x
                # 1.3us DVE normalize ops; they execute during exp(b).
                if pending is not None:
                    pb, pt, prz = pending
                    norm_chunks(pt, prz, 0, 2)
                    norm_chunks(pt, prz, 2, 2)
                    store_chunks(pb, pt, 0, 4)
                    norm_chunks(pt, prz, 4, 2)
                    norm_chunks(pt, prz, 6, 2)
                    store_chunks(pb, pt, 4, 4)
                pending = (b, t_sb, rz)

            # epilogue: last batch in eighths so the first store DMA starts
            # as early as possible after rz
            pb, pt, prz = pending
            for c in range(NQ):
                norm_chunks(pt, prz, c, 1)
                store_chunks(pb, pt, c, 1)

    nc.compile()
    return nc


_CACHED_NC = None


def kernel(**inputs: np.ndarray) -> np.ndarray:
    global _CACHED_NC
    query = np.ascontiguousarray(np.asarray(inputs["query"], dtype=np.float32))
    wq = np.ascontiguousarray(np.asarray(inputs["W_query"], dtype=np.float32))
    wk = np.ascontiguousarray(np.asarray(inputs["W_key"], dtype=np.float32))
    assert query.shape == (B, S, D), query.shape

    if _CACHED_NC is None:
        _CACHED_NC = build_bass()
    nc = _CACHED_NC

    in_maps = [
        {
            "query": query[c * BPC:(c + 1) * BPC],
            "W_query": wq,
            "W_key": wk,
        }
        for c in range(N_CORES)
    ]
    res = run_bass_kernel_spmd(nc, in_maps, core_ids=list(range(N_CORES)))
    out = np.concatenate(
        [r["out"].reshape(BPC, S * S) for r in res.results], axis=0
    )
    return out
